# revision 139
# baseline (speedup 1.0000x reference)
"""Causal multi-head attention on 8 Trainium2 NeuronCores.

Sharding: tensor-parallel across heads. 16 heads, 8 cores -> 2 heads/core.
Each core reads the full activations (pre-transposed + pre-cast to bf16 on
the host) and its slice of the projection weights; it emits the partial
output  concat_c @ WoT_c  in bf16 and the host sums the 8 partials (the
"all-reduce after the output projection").

Everything on the matmul path is bf16 (same PE rate as f32r, half the HBM
traffic); PSUM accumulation stays f32.  Per-core HBM traffic is ~33 MiB in
~64 large DMA instructions.

Device schedule (single core):
  - Q/K projections produce QHT/KHT [128 local dims, T] per batch.
  - V is projected directly into row-major VH blocks [128 rows, 64] per
    head (no PE transpose needed): vh = xt_v.T @ WvT per 128-row block,
    with a ones-column appended so the PV matmul also produces softmax
    denominators.
  - Attention per (batch, 512-query group): per key-block pair, both
    heads' ST matmuls write one [128, 1024] PSUM strip, one (or two,
    on diagonal strips) Exp activations produce bf16 P, and the PV
    accumulation lags five pairs so the PE never waits on the ACT
    engine.  Diagonal strips only compute the unmasked column range;
    the triangle block gets a bf16 mask multiply on the DVE.
  - Each group's epilogue (PV flush, reciprocal of the denominator row,
    Pool partition-broadcast, DVE normalize into otall) is deferred into
    the NEXT attention slot's fillers, so the next slot's ST/exp chain
    starts right at the slot boundary; the output projection streams per
    128-row block to DRAM one slot later still.
  - Emission is software-pipelined: the previous slot's epilogue, the
    next group's projection, input DMAs two groups ahead and the output
    projection of the previous group are interleaved as fillers inside
    each attention group's key-block loop, so the PE stays dense and its
    p-state ramp stays at the full 2.4 GHz clock.
  - Engine balance: exp + early projection copies on ACT, late
    projection copies / masks / reciprocal / normalize / output copies
    on DVE, partition-broadcasts on Pool (GPSIMD cannot touch PSUM).
  - The last group runs a fused tail epilogue: per-128-row normalize +
    output projection + store, with reserved outproj chunks burned on
    the PE while the final reciprocal/broadcast chain lands.
"""

import numpy as np

B, T, C = 2, 2048, 1024
H, DK = 16, 64
NCORES = 8
HL = H // NCORES          # local heads per core = 2
LD = HL * DK              # local head dims per core = 128
N = B * T                 # 4096 rows
KCH = C // 128            # 8 contraction chunks
QG = T // 512             # 4 query groups per batch
KB = T // 128             # 16 key blocks per batch

LAST_RESULTS = None       # BassKernelResults of the most recent run (for test.py)


def _build_program():
    import concourse.tile as tile
    import concourse.mybir as mybir
    from concourse import bacc
    from contextlib import ExitStack

    f32 = mybir.dt.float32
    bf16 = mybir.dt.bfloat16
    u16 = mybir.dt.uint16
    EXP = mybir.ActivationFunctionType.Exp

    nc = bacc.Bacc("TRN2", target_bir_lowering=False, debug=False, num_devices=NCORES)
    qT_d = nc.declare_dram_parameter("qT", [128, KCH, N], bf16, isOutput=False)
    kT_d = nc.declare_dram_parameter("kT", [128, KCH, N], bf16, isOutput=False)
    vT_d = nc.declare_dram_parameter("vT", [128, KCH, N], bf16, isOutput=False)
    wq_d = nc.declare_dram_parameter("wqT", [128, KCH, LD], bf16, isOutput=False)
    wk_d = nc.declare_dram_parameter("wkT", [128, KCH, LD], bf16, isOutput=False)
    wv_d = nc.declare_dram_parameter("wvT", [128, KCH, LD], bf16, isOutput=False)
    wo_d = nc.declare_dram_parameter("woT", [LD, C], bf16, isOutput=False)
    mk_d = nc.declare_dram_parameter("masks", [128, 128], bf16, isOutput=False)
    out_d = nc.declare_dram_parameter("out", [N, C], bf16, isOutput=True)

    with ExitStack() as ctx:
        tc = ctx.enter_context(tile.TileContext(nc))
        const = ctx.enter_context(tc.tile_pool(name="const", bufs=1))
        persist = ctx.enter_context(tc.tile_pool(name="persist", bufs=1))
        xpool = ctx.enter_context(tc.tile_pool(name="xt", bufs=3))
        ppool = ctx.enter_context(tc.tile_pool(name="p", bufs=14))
        opool = ctx.enter_context(tc.tile_pool(name="ot", bufs=24))
        spool = ctx.enter_context(tc.tile_pool(name="small", bufs=2))
        mmps = ctx.enter_context(tc.tile_pool(name="mmps", bufs=2, space="PSUM"))
        otps = ctx.enter_context(tc.tile_pool(name="otps", bufs=2, space="PSUM"))

        # ---- constants / weights (one DMA each) ----
        wq = const.tile([128, KCH, LD], bf16, name="wq")
        wk = const.tile([128, KCH, LD], bf16, name="wk")
        wv = const.tile([128, KCH, LD], bf16, name="wv")
        wo = const.tile([128, C], bf16, name="wo")
        masks = const.tile([128, 128], bf16, name="masks")


        # per-batch persistent activations
        qht = [persist.tile([128, T], bf16, name=f"qht{b}") for b in range(B)]
        kht = [persist.tile([128, T], bf16, name=f"kht{b}") for b in range(B)]
        otall = [persist.tile([128, T], bf16, name=f"otall{b}") for b in range(B)]
        # VH blocks [128 rows, 65] per (batch, local head); col 64 = ones
        vh = [[persist.tile([128, KB, 65], bf16, name=f"vh{b}{l}")
               for l in range(HL)] for b in range(B)]
        for b in range(B):
            for l in range(HL):
                nc.vector.memset(vh[b][l][:, :, 64:65].bitcast(u16), 16256)  # 1.0bf16

        xts = {}   # (b, n) -> {"k": tile, "q": tile, "v": tile}

        def dma_one(g, t, split=False):
            b, n = g
            src = {"k": kT_d, "q": qT_d, "v": vT_d}[t]
            xt = xpool.tile([128, KCH, 512], bf16, tag=f"x{t}", name=f"x{t}_{b}_{n}")
            cols = slice(b * T + n * 512, b * T + (n + 1) * 512)
            if split:
                nc.sync.dma_start(xt[:, 0:4, :], src[:, 0:4, cols])
                nc.sync.dma_start(xt[:, 4:8, :], src[:, 4:8, cols])
            else:
                nc.sync.dma_start(xt[:], src[:, :, cols])
            xts.setdefault(g, {})[t] = xt

        def proj_chunks(g):
            # small closures: k-proj (2), k-copy, q-proj (2), q-copy,
            # v-proj (4 row blocks), v-copies
            b, n = g
            st = {}

            def kq_mm(t, w_t, half):
                def go():
                    key = f"ps_{t}"
                    if half == 0:
                        st[key] = mmps.tile([128, 512], f32, tag="proj",
                                            name=f"ps{t}_{b}_{n}")
                    ps = st[key]
                    for kk in range(4 * half, 4 * half + 4):
                        nc.tensor.matmul(ps[:], w_t[:, kk, :], xts[g][t][:, kk, :],
                                         start=(kk == 0), stop=(kk == KCH - 1))
                return go

            def kq_copy(t, dst, eng):
                def go():
                    with nc.allow_low_precision(reason="bf16 activations"):
                        if eng is nc.scalar:
                            eng.copy(dst[:, n * 512:(n + 1) * 512], st[f"ps_{t}"][:])
                        else:
                            eng.tensor_copy(dst[:, n * 512:(n + 1) * 512],
                                            st[f"ps_{t}"][:])
                return go

            def v_mm(rb):
                def go():
                    if rb == 0:
                        st["ps_v"] = mmps.tile([128, 4, 128], f32, tag="proj",
                                               name=f"psv_{b}_{n}")
                    ps = st["ps_v"]
                    for kk in range(KCH):
                        nc.tensor.matmul(ps[:, rb, :],
                                         xts[g]["v"][:, kk, rb * 128:(rb + 1) * 128],
                                         wv[:, kk, :],
                                         start=(kk == 0), stop=(kk == KCH - 1))
                return go

            def v_copy(l):
                def go():
                    with nc.allow_low_precision(reason="bf16 activations"):
                        if n <= 1:
                            nc.scalar.copy(vh[b][l][:, 4 * n:4 * n + 4, 0:64],
                                           st["ps_v"][:, :, l * 64:l * 64 + 64])
                        else:
                            nc.vector.tensor_copy(
                                vh[b][l][:, 4 * n:4 * n + 4, 0:64],
                                st["ps_v"][:, :, l * 64:l * 64 + 64])
                return go

            ceng = nc.scalar if n <= 2 else nc.vector
            return ([kq_mm("k", wk, 0), kq_mm("k", wk, 1),
                     kq_copy("k", kht[b], ceng),
                     kq_mm("q", wq, 0), kq_mm("q", wq, 1),
                     kq_copy("q", qht[b], ceng)]
                    + [v_mm(rb) for rb in range(4)]
                    + [v_copy(l) for l in range(HL)])

        def outproj_chunks(g, defer=None):
            # when defer is a list, the output DMA closures are appended to
            # it instead of being emitted inline -- lets the schedule hold
            # early output DMAs until the last input DMAs have been issued,
            # so stores never preempt the input stream on the DMA engines.
            b, qg = g

            def go_rt(rt):
                def go():
                    row0 = qg * 512 + rt * 128
                    ot = opool.tile([128, C], bf16, tag="ot", name=f"ot_{b}_{qg}_{rt}")
                    for nn in range(2):
                        ops = mmps.tile([128, 512], f32, tag="proj",
                                        name=f"ops_{b}_{qg}_{rt}_{nn}")
                        nc.tensor.matmul(ops[:], otall[b][:, row0:row0 + 128],
                                         wo[:, nn * 512:(nn + 1) * 512],
                                         start=True, stop=True)
                        with nc.allow_low_precision(reason="bf16 output"):
                            nc.vector.tensor_copy(
                                ot[:, nn * 512:(nn + 1) * 512], ops[:])
                    def dma():
                        nc.sync.dma_start(
                            out_d[b * T + row0: b * T + row0 + 128, :], ot[:])
                    if defer is None:
                        dma()
                    else:
                        defer.append(dma)
                return go
            return [go_rt(rt) for rt in range(4)]

        def attention(g, fillers, tail=False, reserve=(), nlag=5):
            b, qg = g
            q0 = qg * 512
            nkb = 4 * qg + 4
            npair = nkb // 2
            otp = [otps.tile([65, 512], f32, tag="otp", name=f"otp_{b}_{qg}_{l}")
                   for l in range(HL)]
            pend = [[], []]   # per head: list of (p_tile, half, kb, c0)
            fq = list(fillers)
            popped = 0

            def pv(l, p_t, half, kb, c0):
                nc.tensor.matmul(otp[l][:, c0:512], vh[b][l][:, kb, :],
                                 p_t[:, 512 * half + c0: 512 * half + 512],
                                 start=(kb == 0), stop=(kb == nkb - 1))

            for pair in range(npair):
                kb0, kb1 = 2 * pair, 2 * pair + 1
                d0, d1 = kb0 - 4 * qg, kb1 - 4 * qg
                c00 = 128 * d0 if d0 > 0 else 0
                c01 = 128 * d1 if d1 > 0 else 0
                for l in range(HL):
                    hs = slice(l * 64, (l + 1) * 64)
                    stt = mmps.tile([128, 1024], f32, tag="mm",
                                    name=f"st_{b}_{qg}_{l}_{pair}")
                    p_t = ppool.tile([128, 1024], bf16, tag="p",
                                     name=f"p_{b}_{qg}_{l}_{pair}")
                    for h, kb, c0 in ((0, kb0, c00), (1, kb1, c01)):
                        nc.tensor.matmul(stt[:, 512 * h + c0: 512 * h + 512],
                                         kht[b][hs, kb * 128:(kb + 1) * 128],
                                         qht[b][hs, q0 + c0: q0 + 512],
                                         start=True, stop=True)
                    if d1 < 0:
                        # both halves below the diagonal: one full-width exp
                        nc.scalar.activation(p_t[:], stt[:], EXP, scale=0.125)
                    else:
                        nc.scalar.activation(p_t[:, c00:512], stt[:, c00:512],
                                             EXP, scale=0.125)
                        nc.scalar.activation(p_t[:, 512 + c01:1024],
                                             stt[:, 512 + c01:1024],
                                             EXP, scale=0.125)
                    for h, kb, c0, d in ((0, kb0, c00, d0), (1, kb1, c01, d1)):
                        if d >= 0:
                            # triangle mask on the diagonal 128-block
                            off = 512 * h + c0
                            with nc.allow_low_precision(reason="bf16 P"):
                                nc.vector.tensor_mul(p_t[:, off:off + 128],
                                                     p_t[:, off:off + 128], masks[:])
                        pend[l].append((p_t, h, kb, c0))
                # PV lags five pairs (three in the tail slot, so the
                # final flush before the reciprocal chain is short)
                lag = 3 if tail else nlag
                if pair >= lag:
                    for l in range(HL):
                        for (p_t, h, kb, c0) in pend[l][:2]:
                            pv(l, p_t, h, kb, c0)
                        del pend[l][:2]
                # drain fillers evenly across pair iterations (front-loaded
                # in the tail slot, where every filler is long since ready)
                horizon = npair
                want = len(fq) * min(pair + 1, horizon) // horizon
                while popped < want:
                    fq[popped]()
                    popped += 1
            while popped < len(fq):
                fq[popped]()
                popped += 1

            def finish():
                # flush per head: head 0's reciprocal overlaps head 1's
                # remaining PV matmuls on the PE
                recips, reps = [], []
                for l in range(HL):
                    for (p_t, h, kb, c0) in pend[l]:
                        pv(l, p_t, h, kb, c0)
                    pend[l] = []
                    recip = spool.tile([1, 512], f32, tag="recip",
                                       name=f"rc_{b}_{qg}_{l}")
                    nc.vector.reciprocal(recip[:], otp[l][64:65, :])
                    recips.append(recip)
                if not tail:
                    for l in range(HL):
                        rep = spool.tile([64, 512], f32, tag="rep",
                                         name=f"rp_{b}_{qg}_{l}")
                        nc.gpsimd.partition_broadcast(rep[:], recips[l][:])
                        reps.append(rep)
                    for half in range(2):
                        hc = slice(half * 256, half * 256 + 256)
                        for l in range(HL):
                            hs = slice(l * 64, (l + 1) * 64)
                            with nc.allow_low_precision(reason="bf16 attn out"):
                                nc.vector.tensor_mul(
                                    otall[b][hs, q0 + half * 256:
                                             q0 + half * 256 + 256],
                                    otp[l][0:64, hc], reps[l][:, hc])
                    return
                # tail epilogue: normalize + project out per 128-row chunk so
                # the final output DMAs start as early as possible.
                for l in range(HL):
                    rep = spool.tile([64, 512], f32, tag="rep",
                                     name=f"rp_{b}_{qg}_{l}")
                    nc.gpsimd.partition_broadcast(rep[:], recips[l][:])
                    reps.append(rep)
                for ch in reserve:   # ready PE work to burn while the norm
                    ch()             # chain's reciprocal/broadcast land
                for rt in range(4):
                    cs = slice(q0 + rt * 128, q0 + rt * 128 + 128)
                    for l in range(HL):
                        hs = slice(l * 64, (l + 1) * 64)
                        with nc.allow_low_precision(reason="bf16 attn output"):
                            nc.vector.tensor_mul(otall[b][hs, cs], otp[l][0:64,
                                                 rt * 128:(rt + 1) * 128],
                                                 reps[l][:, rt * 128:(rt + 1) * 128])
                    row0 = q0 + rt * 128
                    ot = opool.tile([128, C], bf16, tag="ot",
                                    name=f"ot_{b}_{qg}_{rt}")
                    ops = mmps.tile([128, 1024], f32, tag="mm", name=f"opst_{rt}")
                    for nn in range(2):
                        nc.tensor.matmul(ops[:, nn * 512:(nn + 1) * 512],
                                         otall[b][:, row0:row0 + 128],
                                         wo[:, nn * 512:(nn + 1) * 512],
                                         start=True, stop=True)
                        with nc.allow_low_precision(reason="bf16 output"):
                            if nn == 0 or rt == 3:
                                nc.scalar.copy(ot[:, nn * 512:(nn + 1) * 512],
                                               ops[:, nn * 512:(nn + 1) * 512])
                            else:
                                nc.vector.tensor_copy(
                                    ot[:, nn * 512:(nn + 1) * 512],
                                    ops[:, nn * 512:(nn + 1) * 512])
                        if rt == 3:
                            # last block: fire each half as its copy lands
                            nc.sync.dma_start(
                                out_d[b * T + row0: b * T + row0 + 128,
                                      nn * 512:(nn + 1) * 512],
                                ot[:, nn * 512:(nn + 1) * 512])
                    if rt < 3:
                        nc.sync.dma_start(
                            out_d[b * T + row0: b * T + row0 + 128, :], ot[:])

            if tail:
                finish()
                return None
            return finish

        # ---- schedule ----
        # attention slots alternate batches, ascending query group; each
        # slot's fillers run the next slot's projection, the input DMAs two
        # slots ahead, and the previous slot's output projection.
        A = [(b, n) for n in range(QG) for b in range(B)]

        # prologue: weights + first two groups' inputs, PE starts ASAP
        nc.sync.dma_start(wk[:], wk_d[:])
        dma_one(A[0], "k", split=True)
        nc.sync.dma_start(wq[:], wq_d[:])
        dma_one(A[0], "q", split=True)
        nc.sync.dma_start(wv[:], wv_d[:])
        dma_one(A[0], "v")
        dma_one(A[1], "k")
        dma_one(A[1], "q")
        nc.sync.dma_start(masks[:], mk_d[:])
        nc.sync.dma_start(wo[:], wo_d[:])
        dma_one(A[1], "v")

        pg0 = proj_chunks(A[0])
        for ch in pg0[:6]:     # k,q projection of group (0,0)
            ch()

        pg1 = proj_chunks(A[1])
        fill0 = (pg0[6:] + pg1
                 + [lambda t=t: dma_one(A[2], t) for t in ("k", "q", "v")])
        # each slot's flush+normalize (finish) is deferred into the NEXT
        # slot's fillers, so the next slot's first ST/exp chain starts on
        # the PE/ACT immediately at the slot boundary
        fin = attention(A[0], fill0)
        held_dmas = []
        for i in range(1, 8):
            fill = [fin] if fin is not None else []
            if i + 1 < 8:
                fill += proj_chunks(A[i + 1])
            if i + 2 < 8:
                fill += [lambda t=t, g=A[i + 2]: dma_one(g, t)
                         for t in ("k", "q", "v")]
            if i <= 4:
                op = outproj_chunks(A[i - 1], defer=held_dmas)
            else:
                op = outproj_chunks(A[i - 1])
            if i == 4:
                fill += held_dmas[:8]
            elif i == 5:
                fill += held_dmas[8:]
            if i == 7:
                fill += op[:2]
                fin = attention(A[i], fill, tail=True, reserve=op[2:])
            else:
                fill += op
                fin = attention(A[i], fill)

    nc.compile()
    return nc


def _make_masks():
    j = np.arange(128)[None, :]
    p = np.arange(128)[:, None]
    return (j >= p).astype(np.float32)


def kernel(q, k, v, Wq, Wk, Wv, Wo):
    global LAST_RESULTS
    import ml_dtypes
    from concourse.bass_utils import run_bass_kernel_spmd

    bf = ml_dtypes.bfloat16

    def to3d(x):
        # [B,T,C] f32 -> [128, KCH, N] bf16 with [p, kk, col] = x.T[kk*128+p, col]
        x2 = np.asarray(x, np.float32).reshape(N, KCH, 128).transpose(2, 1, 0)
        return np.ascontiguousarray(x2.astype(bf))

    qT = to3d(q)
    kT = to3d(k)
    vT = to3d(v)
    Wq = np.asarray(Wq, np.float32)
    Wk = np.asarray(Wk, np.float32)
    Wv = np.asarray(Wv, np.float32)
    Wo = np.asarray(Wo, np.float32)
    masks = _make_masks().astype(bf)

    def wslice(W, c):
        # [128, KCH, LD] bf16 with [p, kk, m] = W[c*LD:(c+1)*LD, :].T[kk*128+p, m]
        A = W[c * LD:(c + 1) * LD, :].T
        return np.ascontiguousarray(
            A.reshape(KCH, 128, LD).transpose(1, 0, 2).astype(bf))

    in_maps = []
    for c in range(NCORES):
        sl = slice(c * LD, (c + 1) * LD)
        in_maps.append({
            "qT": qT, "kT": kT, "vT": vT,
            "wqT": wslice(Wq, c),
            "wkT": wslice(Wk, c),
            "wvT": wslice(Wv, c),
            "woT": np.ascontiguousarray(Wo[:, sl].T.astype(bf)),
            "masks": masks,
        })

    nc = _build_program()
    res = run_bass_kernel_spmd(nc, in_maps, list(range(NCORES)))
    LAST_RESULTS = res
    acc = np.zeros((N, C), np.float64)
    for rmap in res.results:
        acc += np.asarray(rmap["out"], np.float64)
    return acc.astype(np.float32).reshape(B, T, C)
